# revision 8
# baseline (speedup 1.0000x reference)
"""RGCN (3-layer, basis-decomposition) message-passing kernel for 8 trn2
NeuronCores.

Strategy (per core k of 8, src-sharded edges, node range I_k):
  - transform: H[(r, src_local), C] message table for the core's src shard,
    stored as two relation-half DRAM tables (rows <= 32767 for int16
    dma_gather indices).
  - edge pass: per dst-tile (128 padded-global rows), chunks of 128 edges:
    bulk dma_gather of message rows, ACT scale by w_e = 1/max(cnt,1), DVE
    is_equal one-hot T[lane, dst_mod_128], PE matmul T.T @ Mw accumulated in
    PSUM, flushed to a partial agg [8*N_pad, C].
  - ReduceScatter(add) the partial aggs -> agg shard [N_pad, C] per core.
  - post: h_l = relu(agg + x @ root + bias) on the node shard; h kept
    transposed in SBUF as lhsT for the next layer's transform.
  - final: concat [h1,h2,h3] -> log_softmax -> output shard.

Host-side prep is integer/index plumbing only (edge bucketing, degree
counts -> 1/cnt weights, schedule capacities, index arrays).
"""
import sys

for _p in ('/opt/trn_rl_repo', '/root/.axon_site/_ro/trn_rl_repo'):
    if _p not in sys.path:
        sys.path.insert(0, _p)

import numpy as np

P = 128          # partitions
NCORES = 8


# ---------------------------------------------------------------------------
# host-side plan
# ---------------------------------------------------------------------------
def _build_plan(edge_index, edge_type, N, R):
    E = edge_index.shape[1]
    src = np.asarray(edge_index[0], dtype=np.int64)
    dst = np.asarray(edge_index[1], dtype=np.int64)
    typ = np.asarray(edge_type, dtype=np.int64)

    n_loc = N // NCORES
    assert n_loc * NCORES == N
    n_pad = ((n_loc + P - 1) // P) * P
    tiles_loc = n_pad // P
    nr_tot = NCORES * n_pad          # padded-global row count
    t_dst = nr_tot // P              # number of dst tiles
    rh = R // 2                      # relations per half

    # per-edge derived quantities
    cnt = np.bincount(typ * N + dst, minlength=R * N)
    w = 1.0 / np.maximum(cnt[typ * N + dst], 1).astype(np.float64)
    core = src // n_loc
    src_loc = src - core * n_loc
    half = (typ >= rh).astype(np.int64)
    grow = (typ % rh) * n_pad + src_loc            # gather row within half-table
    pdst = (dst // n_loc) * n_pad + (dst % n_loc)  # padded-global dst row
    dtile = pdst // P
    dcol = pdst % P

    # bucket edges per (core, dtile, half)
    counts = np.zeros((NCORES, t_dst, 2), dtype=np.int64)
    np.add.at(counts, (core, dtile, half), 1)
    caps = np.maximum(
        (counts + P - 1) // P, 0
    ).max(axis=0)                                   # [t_dst, 2] chunks per tile-half

    tot_chunksA = int(caps[:, 0].sum())
    tot_chunksB = int(caps[:, 1].sum())
    tot_chunks = tot_chunksA + tot_chunksB

    # schedule: per dst tile, list of (half, stream_pos, global_pos)
    sched = []
    gpos = 0
    aposs = 0
    bposs = 0
    for t in range(t_dst):
        entries = []
        for h in (0, 1):
            for _ in range(int(caps[t, h])):
                if h == 0:
                    entries.append((0, aposs, gpos)); aposs += 1
                else:
                    entries.append((1, bposs, gpos)); bposs += 1
                gpos += 1
        sched.append(entries)

    # per-core edge slot assignment
    # sort by (core, dtile, half) to get contiguous segments
    key = ((core * t_dst + dtile) * 2 + half)
    order = np.argsort(key, kind='stable')

    # chunk start offsets in the unified (t,h) chunk order, per half-stream
    a_start = np.zeros(t_dst, dtype=np.int64)   # first A-chunk stream_pos of tile t
    b_start = np.zeros(t_dst, dtype=np.int64)
    a_start[1:] = np.cumsum(caps[:, 0])[:-1]
    b_start[1:] = np.cumsum(caps[:, 1])[:-1]
    g_start = np.zeros((t_dst, 2), dtype=np.int64)  # global chunk id of first chunk
    run = np.cumsum(caps.sum(axis=1))
    g_start[0, 0] = 0
    g_start[:, 0] = np.concatenate([[0], run[:-1]])
    g_start[:, 1] = g_start[:, 0] + caps[:, 0]

    plan = dict(N=N, E=E, R=R, rh=rh, n_loc=n_loc, n_pad=n_pad,
                tiles_loc=tiles_loc, nr_tot=nr_tot, t_dst=t_dst,
                caps=caps, sched=sched, tot_chunks=tot_chunks,
                tot_chunksA=tot_chunksA, tot_chunksB=tot_chunksB)

    # per-core data arrays
    per_core = []
    for k in range(NCORES):
        sel = order[(core[order] == k)]
        # slot arrays sized by schedule capacities
        idx_slots = [np.zeros(tot_chunksA * P, dtype=np.int32),
                     np.zeros(tot_chunksB * P, dtype=np.int32)]
        w_slots = np.zeros(tot_chunks * P, dtype=np.float32)
        col_slots = np.zeros(tot_chunks * P, dtype=np.float32)

        # walk this core's edges grouped by (dtile, half)
        et = dtile[sel]
        eh = half[sel]
        seg_ids = et * 2 + eh
        # boundaries of segments
        uniq, first_idx, seg_cnt = np.unique(seg_ids, return_index=True,
                                             return_counts=True)
        for u, fi, cte in zip(uniq, first_idx, seg_cnt):
            t = int(u) // 2
            h = int(u) % 2
            seg = sel[fi:fi + cte]
            # slots for this (t,h): stream chunks [a_start[t] .. +caps)
            spos0 = (a_start if h == 0 else b_start)[t]
            gpos0 = g_start[t, h]
            n_ch = int(caps[t, h])
            assert cte <= n_ch * P
            slot = np.arange(cte)
            ch = slot // P          # chunk index within segment
            lane = slot % P
            stream_slot = (spos0 + ch) * P + lane
            glob_slot = (gpos0 + ch) * P + lane
            idx_slots[h][stream_slot] = grow[seg]
            w_slots[glob_slot] = w[seg]
            col_slots[glob_slot] = dcol[seg]

        per_core.append(dict(idxA=idx_slots[0], idxB=idx_slots[1],
                             w=w_slots, col=col_slots))
    plan['per_core'] = per_core
    return plan


def _wrap_idx16(idx_flat, n_chunks, strip_chunks):
    """Lay out int16 gather indices for dma_gather: index j of a call goes to
    sbuf [j%16, j//16], replicated across the 8 groups of 16 partitions.
    The stream is split into strips of strip_chunks*128 indices; we emit one
    [128, n_strips*strip_chunks*8] array whose per-strip column slices are the
    per-call index tensors."""
    n_strips = (n_chunks + strip_chunks - 1) // strip_chunks
    tot = n_strips * strip_chunks * P
    buf = np.zeros(tot, dtype=np.int16)
    buf[:len(idx_flat)] = idx_flat.astype(np.int16)
    # per strip: [S*128] -> [S*8, 16] -> transpose -> [16, S*8]
    ncol = strip_chunks * 8
    out = np.zeros((16, n_strips * ncol), dtype=np.int16)
    for s in range(n_strips):
        blk = buf[s * strip_chunks * P:(s + 1) * strip_chunks * P]
        out[:, s * ncol:(s + 1) * ncol] = blk.reshape(ncol, 16).T
    return np.tile(out, (8, 1)), n_strips


def _chunked_lanes(arr_flat, tot_chunks):
    """[tot_chunks*128] -> [128, tot_chunks] with lane p of chunk j at [p, j]."""
    return np.ascontiguousarray(arr_flat.reshape(tot_chunks, P).T)


# ---------------------------------------------------------------------------
# bass program
# ---------------------------------------------------------------------------
def _build_program(plan, STRIP=8):
    import tile_patch  # noqa: F401  (applies walrus wait-split workaround)
    import concourse.bass as bass
    import concourse.mybir as mybir
    import concourse.tile as tile
    from concourse import library_config
    from concourse.masks import make_identity

    f32 = mybir.dt.float32
    i16 = mybir.dt.int16
    AF = mybir.ActivationFunctionType
    ALU = mybir.AluOpType
    AX = mybir.AxisListType

    R = plan['R']; rh = plan['rh']
    n_pad = plan['n_pad']; tiles_loc = plan['tiles_loc']
    nr_tot = plan['nr_tot']; t_dst = plan['t_dst']
    caps = plan['caps']; sched = plan['sched']
    tot_chunks = plan['tot_chunks']
    C = 128
    half_rows = rh * n_pad

    nstripsA = (plan['tot_chunksA'] + STRIP - 1) // STRIP
    nstripsB = (plan['tot_chunksB'] + STRIP - 1) // STRIP
    plan['nstripsA'], plan['nstripsB'] = nstripsA, nstripsB

    nc = bass.Bass()

    # ---- I/O ----
    basis1_loc = nc.dram_tensor("basis1_loc", [4, n_pad, C], f32, kind="ExternalInput")
    root1_loc = nc.dram_tensor("root1_loc", [n_pad, C], f32, kind="ExternalInput")
    w1diag = nc.dram_tensor("w1diag", [4, C, R * C], f32, kind="ExternalInput")
    basis_b0 = nc.dram_tensor("basis_b0", [4, C, C], f32, kind="ExternalInput")
    root_b0 = nc.dram_tensor("root_b0", [C, C], f32, kind="ExternalInput")
    basis_b1 = nc.dram_tensor("basis_b1", [4, 2 * C, C], f32, kind="ExternalInput")
    root_b1 = nc.dram_tensor("root_b1", [2 * C, C], f32, kind="ExternalInput")
    biases = nc.dram_tensor("biases", [1, 3 * C], f32, kind="ExternalInput")
    iota_in = nc.dram_tensor("iota_in", [P, C], f32, kind="ExternalInput")
    idxA_in = nc.dram_tensor("idxA_in", [P, nstripsA * STRIP * 8], i16, kind="ExternalInput")
    idxB_in = nc.dram_tensor("idxB_in", [P, nstripsB * STRIP * 8], i16, kind="ExternalInput")
    col_in = nc.dram_tensor("col_in", [P, tot_chunks], f32, kind="ExternalInput")
    w_in = nc.dram_tensor("w_in", [P, tot_chunks], f32, kind="ExternalInput")
    out_t = nc.dram_tensor("out", [n_pad, 3 * C], f32, kind="ExternalOutput")

    comp1 = plan['comp1']; comp_b0 = plan['comp_b0']; comp_b1 = plan['comp_b1']

    with tile.TileContext(nc) as tc:
        with (
            tc.tile_pool(name="dram", bufs=1, space="DRAM") as dram,
            tc.tile_pool(name="persist", bufs=1) as ps,
            tc.tile_pool(name="sbuf", bufs=2) as sb,
            tc.tile_pool(name="slabA", bufs=3) as slabA_pool,
            tc.tile_pool(name="slabB", bufs=3) as slabB_pool,
            tc.tile_pool(name="psum_tp", bufs=2, space="PSUM") as ptp,
            tc.tile_pool(name="psum_ep", bufs=2, space="PSUM") as pep,
        ):
            nc.gpsimd.load_library(library_config.mlp)

            # DRAM intermediates
            H_A = [dram.tile([half_rows, C], f32, name=f"HA{l}", tag=f"HA{l}") for l in range(3)]
            H_B = [dram.tile([half_rows, C], f32, name=f"HB{l}", tag=f"HB{l}") for l in range(3)]
            aggbuf = [dram.tile([nr_tot, C], f32, name=f"agg{l}", tag=f"agg{l}") for l in range(3)]
            agg_red = [dram.tile([n_pad, C], f32, name=f"aggr{l}", tag=f"aggr{l}") for l in range(3)]

            # persistent SBUF
            identity = ps.tile([P, P], f32)
            make_identity(nc, identity[:])
            iota = ps.tile([P, C], f32)
            nc.sync.dma_start(iota[:], iota_in[:])
            ones_row = ps.tile([1, P], f32)
            nc.vector.memset(ones_row[:], 1.0)
            bias_rows = ps.tile([1, 3 * C], f32)
            nc.sync.dma_start(bias_rows[:], biases[:])
            colv = ps.tile([P, tot_chunks], f32)
            nc.sync.dma_start(colv[:], col_in[:])
            wv = ps.tile([P, tot_chunks], f32)
            nc.sync.dma_start(wv[:], w_in[:])
            idxA = ps.tile([P, nstripsA * STRIP * 8], i16)
            nc.sync.dma_start(idxA[:], idxA_in[:])
            idxB = ps.tile([P, nstripsB * STRIP * 8], i16)
            nc.sync.dma_start(idxB[:], idxB_in[:])

            xT1 = ps.tile([P, n_pad], f32)   # h1 transposed (c on partitions)
            xT2 = ps.tile([P, n_pad], f32)
            h3s = ps.tile([P, n_pad], f32)   # h3 in [i, c] layout, tiles along free

            # relation weight matrices for layers 2/3, built on DVE
            w2 = ps.tile([P, R * C], f32)
            w3a = ps.tile([P, R * C], f32)
            w3b = ps.tile([P, R * C], f32)
            rootb0 = ps.tile([P, C], f32)
            nc.sync.dma_start(rootb0[:], root_b0[:])
            rootb1a = ps.tile([P, C], f32)
            nc.sync.dma_start(rootb1a[:], root_b1[0:C, :])
            rootb1b = ps.tile([P, C], f32)
            nc.sync.dma_start(rootb1b[:], root_b1[C:2 * C, :])

            def build_w(dst_tile, basis_dram, comp, row0):
                bt = sb.tile([P, 4 * C], f32, tag="wbuild")
                nc.sync.dma_start(
                    bt[:].rearrange("p (b c) -> p b c", c=C),
                    basis_dram[:, row0:row0 + P, :].rearrange("b p c -> p b c"))
                tmp = sb.tile([P, C], f32, tag="wtmp")
                for r in range(R):
                    dstap = dst_tile[:, r * C:(r + 1) * C]
                    nc.vector.tensor_scalar_mul(dstap, bt[:, 0:C], float(comp[r, 0]))
                    for b in range(1, 4):
                        nc.vector.tensor_scalar_mul(
                            tmp[:], bt[:, b * C:(b + 1) * C], float(comp[r, b]))
                        nc.vector.tensor_add(dstap, dstap, tmp[:])

            build_w(w2, basis_b0, comp_b0, 0)
            build_w(w3a, basis_b1, comp_b1, 0)
            build_w(w3b, basis_b1, comp_b1, P)

            # -------- per-layer pieces --------
            def transform_l1():
                with tc.tile_pool(name="w1pool", bufs=1) as w1p:
                    w1sb = w1p.tile([P, 4 * R * C], f32)
                    nc.sync.dma_start(
                        w1sb[:].rearrange("p (b n) -> p b n", b=4),
                        w1diag[:].rearrange("b p n -> p b n"))
                    for it in range(tiles_loc):
                        bt = sb.tile([P, 4 * C], f32, tag="l1b")
                        nc.sync.dma_start(
                            bt[:].rearrange("p (b c) -> p b c", c=C),
                            basis1_loc[:, it * P:(it + 1) * P, :].rearrange("b p c -> p b c"))
                        ptr = ptp.tile([P, 4 * C], f32, tag="tpt")
                        for b in range(4):
                            nc.tensor.transpose(
                                ptr[:, b * C:(b + 1) * C], bt[:, b * C:(b + 1) * C],
                                identity[:])
                        btT = sb.tile([P, 4 * C], f32, tag="l1bt")
                        nc.vector.tensor_copy(btT[:], ptr[:])
                        ph = ptp.tile([P, R * C], f32, tag="l1h")
                        for nblk in range(2):
                            cols = slice(nblk * 512, (nblk + 1) * 512)
                            for b in range(4):
                                nc.tensor.matmul(
                                    ph[:, cols], btT[:, b * C:(b + 1) * C],
                                    w1sb[:, b * R * C + nblk * 512:
                                         b * R * C + (nblk + 1) * 512],
                                    start=(b == 0), stop=(b == 3))
                        hs = sb.tile([P, R * C], f32, tag="hslab")
                        nc.vector.tensor_copy(hs[:], ph[:])
                        _write_H(hs, 0, it)

            def _write_H(hs, l, it):
                # hs [128, 8*C]: cols (r, c); rows of half tables
                for h, Ht in ((0, H_A[l]), (1, H_B[l])):
                    nc.sync.dma_start(
                        Ht[:].rearrange("(r n) c -> r n c", r=rh)
                          [:, it * P:(it + 1) * P, :].rearrange("r p c -> p r c"),
                        hs[:, h * rh * C:(h + 1) * rh * C]
                          .rearrange("p (r c) -> p r c", c=C))

            def transform_l23(l):
                # l = 1 (uses xT1, w2) or 2 (uses xT1+xT2, w3a/w3b)
                for it in range(tiles_loc):
                    isl = slice(it * P, (it + 1) * P)
                    ph = ptp.tile([P, R * C], f32, tag="l1h")
                    for r in range(R):
                        cols = slice(r * C, (r + 1) * C)
                        if l == 1:
                            nc.tensor.matmul(ph[:, cols], xT1[:, isl], w2[:, cols],
                                             start=True, stop=True)
                        else:
                            nc.tensor.matmul(ph[:, cols], xT1[:, isl], w3a[:, cols],
                                             start=True, stop=False)
                            nc.tensor.matmul(ph[:, cols], xT2[:, isl], w3b[:, cols],
                                             start=False, stop=True)
                    hs = sb.tile([P, R * C], f32, tag="hslab")
                    nc.vector.tensor_copy(hs[:], ph[:])
                    _write_H(hs, l, it)

            nidx_reg_box = []

            def edge_pass(l):
                stripA = [None] * nstripsA
                stripB = [None] * nstripsB
                if not nidx_reg_box:
                    nidx_reg_box.append(nc.gpsimd.to_reg(STRIP * P))
                nidx_reg = nidx_reg_box[0]

                def get_strip(h, s):
                    cache, pool, idxt, Ht, nst = (
                        (stripA, slabA_pool, idxA, H_A[l], nstripsA) if h == 0
                        else (stripB, slabB_pool, idxB, H_B[l], nstripsB))
                    if cache[s] is None:
                        slab = pool.tile([P, STRIP * C], f32, tag="slab")
                        nc.gpsimd.dma_gather(
                            slab[:].rearrange("p (k c) -> p k c", c=C),
                            Ht[:], idxt[:, s * STRIP * 8:(s + 1) * STRIP * 8],
                            STRIP * P, nidx_reg, C)
                        cache[s] = slab
                    return cache[s]

                flush = sb.tile([P, 8 * C], f32, tag="flush")
                for t in range(t_dst):
                    entries = sched[t]
                    slot = t % 8
                    if not entries:
                        nc.vector.memset(flush[:, slot * C:(slot + 1) * C], 0.0)
                    else:
                        pagg = pep.tile([P, C], f32, tag="ep")
                        for j, (h, spos, gpos) in enumerate(entries):
                            s, off = spos // STRIP, spos % STRIP
                            slab = get_strip(h, s)
                            mw = sb.tile([P, C], f32, tag="mw")
                            nc.scalar.activation(
                                mw[:], slab[:, off * C:(off + 1) * C], AF.Copy,
                                scale=wv[:, gpos:gpos + 1])
                            tt = sb.tile([P, C], f32, tag="onehot")
                            nc.vector.tensor_tensor(
                                out=tt[:],
                                in0=colv[:, gpos:gpos + 1].to_broadcast([P, C]),
                                in1=iota[:], op=ALU.is_equal)
                            nc.tensor.matmul(pagg[:], tt[:], mw[:],
                                             start=(j == 0),
                                             stop=(j == len(entries) - 1))
                        nc.vector.tensor_copy(flush[:, slot * C:(slot + 1) * C],
                                              pagg[:])
                    if slot == 7:
                        t0 = t - 7
                        nc.sync.dma_start(
                            aggbuf[l][t0 * P:(t + 1) * P, :]
                            .rearrange("(t p) c -> p t c", p=P),
                            flush[:].rearrange("p (t c) -> p t c", c=C))
                        flush = sb.tile([P, 8 * C], f32, tag="flush")

                nc.gpsimd.collective_compute(
                    "ReduceScatter", ALU.add,
                    replica_groups=[list(range(NCORES))],
                    ins=[aggbuf[l].opt()], outs=[agg_red[l].opt()])

            def post(l):
                # h_l = relu(agg + x@root + bias); store transposed (l<2) or flat
                for it in range(tiles_loc):
                    isl = slice(it * P, (it + 1) * P)
                    prt = pep.tile([P, C], f32, tag="ep")
                    nc.tensor.matmul(prt[:], ones_row[:],
                                     bias_rows[:, l * C:(l + 1) * C],
                                     start=True, stop=(l == 0))
                    if l == 1:
                        nc.tensor.matmul(prt[:], xT1[:, isl], rootb0[:],
                                         start=False, stop=True)
                    elif l == 2:
                        nc.tensor.matmul(prt[:], xT1[:, isl], rootb1a[:],
                                         start=False, stop=False)
                        nc.tensor.matmul(prt[:], xT2[:, isl], rootb1b[:],
                                         start=False, stop=True)
                    ag = sb.tile([P, C], f32, tag="agt")
                    nc.sync.dma_start(ag[:], agg_red[l][isl, :])
                    t1 = sb.tile([P, C], f32, tag="post1")
                    nc.vector.tensor_add(t1[:], ag[:], prt[:])
                    if l == 0:
                        rt = sb.tile([P, C], f32, tag="rt1")
                        nc.sync.dma_start(rt[:], root1_loc[isl, :])
                        nc.vector.tensor_add(t1[:], t1[:], rt[:])
                    h = sb.tile([P, C], f32, tag="hpost")
                    nc.scalar.activation(h[:], t1[:], AF.Relu)
                    if l < 2:
                        ptr = pep.tile([P, C], f32, tag="ep")
                        nc.tensor.transpose(ptr[:], h[:], identity[:])
                        dst = xT1 if l == 0 else xT2
                        nc.vector.tensor_copy(dst[:, isl], ptr[:])
                    else:
                        nc.vector.tensor_copy(h3s[:, isl], h[:])

            def final():
                for it in range(tiles_loc):
                    isl = slice(it * P, (it + 1) * P)
                    cat = sb.tile([P, 3 * C], f32, tag="cat")
                    for j, xt in enumerate((xT1, xT2)):
                        ptr = pep.tile([P, C], f32, tag="ep")
                        nc.tensor.transpose(ptr[:], xt[:, isl], identity[:])
                        nc.vector.tensor_copy(cat[:, j * C:(j + 1) * C], ptr[:])
                    nc.vector.tensor_copy(cat[:, 2 * C:3 * C], h3s[:, isl])
                    negmax = sb.tile([P, 1], f32, tag="negmax")
                    nc.vector.reduce_max(negmax[:], cat[:], axis=AX.X, negate=True)
                    ex = sb.tile([P, 3 * C], f32, tag="ex")
                    nc.scalar.activation(ex[:], cat[:], AF.Exp, bias=negmax[:])
                    ssum = sb.tile([P, 1], f32, tag="ssum")
                    nc.vector.reduce_sum(ssum[:], ex[:], axis=AX.X)
                    lsum = sb.tile([P, 1], f32, tag="lsum")
                    nc.scalar.activation(lsum[:], ssum[:], AF.Ln)
                    res = sb.tile([P, 3 * C], f32, tag="res")
                    nc.vector.tensor_scalar(res[:], cat[:], negmax[:], lsum[:],
                                            op0=ALU.add, op1=ALU.subtract)
                    nc.sync.dma_start(out_t[isl, :], res[:])

            # -------- layer schedule --------
            transform_l1()
            edge_pass(0)
            post(0)
            transform_l23(1)
            edge_pass(1)
            post(1)
            transform_l23(2)
            edge_pass(2)
            post(2)
            final()

    import tile_patch as tp
    tp.finalize(nc)
    return nc


# ---------------------------------------------------------------------------
# entry point
# ---------------------------------------------------------------------------
def kernel(edge_index, edge_type, basis1, comp1, root1, bias1,
           basis_b0, comp_b0, root_b0, bias_b0,
           basis_b1, comp_b1, root_b1, bias_b1):
    from concourse.bass_utils import run_bass_kernel_spmd

    basis1 = np.asarray(basis1, dtype=np.float32)
    comp1 = np.asarray(comp1, dtype=np.float32)
    root1 = np.asarray(root1, dtype=np.float32)
    N, C = root1.shape
    B = basis1.shape[0]
    R = comp1.shape[0]
    assert C == 128 and B == 4 and R == 8

    plan = _build_plan(np.asarray(edge_index), np.asarray(edge_type), N, R)
    plan['comp1'] = comp1
    plan['comp_b0'] = np.asarray(comp_b0, dtype=np.float32)
    plan['comp_b1'] = np.asarray(comp_b1, dtype=np.float32)

    STRIP = 8
    nc = _build_program(plan, STRIP=STRIP)

    n_loc, n_pad = plan['n_loc'], plan['n_pad']
    # W1diag [4, C, R*C]: W1diag[b, c', r*C + c] = comp1[r, b] * (c == c')
    w1d = np.zeros((4, C, R * C), np.float32)
    eye = np.eye(C, dtype=np.float32)
    for b in range(4):
        for r in range(R):
            w1d[b, :, r * C:(r + 1) * C] = comp1[r, b] * eye
    iota_arr = np.broadcast_to(np.arange(C, dtype=np.float32), (P, C)).copy()
    biases_arr = np.concatenate([np.asarray(bias1, np.float32),
                                 np.asarray(bias_b0, np.float32),
                                 np.asarray(bias_b1, np.float32)])[None, :]

    in_maps = []
    for k in range(NCORES):
        pc = plan['per_core'][k]
        b1l = np.zeros((4, n_pad, C), np.float32)
        b1l[:, :n_loc] = basis1[:, k * n_loc:(k + 1) * n_loc, :]
        r1l = np.zeros((n_pad, C), np.float32)
        r1l[:n_loc] = root1[k * n_loc:(k + 1) * n_loc, :]
        idxA_arr, _ = _wrap_idx16(pc['idxA'], plan['tot_chunksA'], STRIP)
        idxB_arr, _ = _wrap_idx16(pc['idxB'], plan['tot_chunksB'], STRIP)
        in_maps.append({
            "basis1_loc": b1l,
            "root1_loc": r1l,
            "w1diag": w1d,
            "basis_b0": np.asarray(basis_b0, np.float32),
            "root_b0": np.asarray(root_b0, np.float32),
            "basis_b1": np.asarray(basis_b1, np.float32),
            "root_b1": np.asarray(root_b1, np.float32),
            "biases": biases_arr,
            "iota_in": iota_arr,
            "idxA_in": idxA_arr,
            "idxB_in": idxB_arr,
            "col_in": _chunked_lanes(pc['col'], plan['tot_chunks']),
            "w_in": _chunked_lanes(pc['w'], plan['tot_chunks']),
        })

    import os
    trace = os.environ.get('GNN_TRACE', '0') == '1'
    res = run_bass_kernel_spmd(nc, in_maps, list(range(NCORES)), trace=trace)
    global LAST_EXEC_NS, LAST_RESULT
    LAST_EXEC_NS = res.exec_time_ns
    LAST_RESULT = res
    out = np.concatenate(
        [res.results[k]["out"][:n_loc] for k in range(NCORES)], axis=0)
    return out.astype(np.float32)
